# revision 4
# baseline (speedup 1.0000x reference)
"""Trainium2 Bass kernel for 16-head MHA (b=2, n=2048, c=1024, d=64).

Reference semantics (note the inverted scale "bug" reproduced faithfully):
    qkv = x @ W_qkv + b_qkv
    scores = (q @ k^T) * sqrt(d)          # multiplied, not divided
    out = softmax(scores) @ v
    y = concat_heads(out) @ W_proj + b_proj

Sharding: tensor-parallel over heads. Each of the 8 cores computes QKV +
attention for its 2 heads (head-dim-transposed layouts so no activation
transposes are needed beyond one x^T pass), then an AllToAll moves the
per-head attention outputs into a row-sharded layout and each core computes
the final projection for its 512-row output shard. Host concatenates shards.

Softmax: scores are computed twice (two orientations). The natural
orientation ([n_q, n_k]) yields per-row max m and logsumexp via free-dim
reductions; the transposed orientation feeds A@V directly. The combined
bias -(m + ln sum) is folded into the transposed score matmul as an extra
contraction row (ones in lhsT, bias in rhs), so exp(psum) is already the
normalized attention weight.
"""

import sys
from contextlib import ExitStack

sys.path.insert(0, "/opt/trn_rl_repo")

import numpy as np

import concourse.bass as bass
import concourse.tile as tile
from concourse import bacc, mybir
from concourse import bass_utils
from concourse.masks import make_identity

# Problem shape (hardcoded per contract)
B, N, C = 2, 2048, 1024
H, D = 16, 64
NCORES = 8
HPC = H // NCORES          # heads per core = 2
D2 = HPC * D               # 128 = per-core slice of the concat dim
R = B * N                  # 4096 flattened rows
RS = R // NCORES           # 512 output rows per core
KS = C // 128              # 8 contraction blocks of 128
CHUNK = 512                # rows per x^T/qkv chunk
NCH = R // CHUNK           # 8 chunks
NQT = N // 128             # 16 query tiles per batch
NKT = N // 128             # 16 key tiles per batch
F32 = mybir.dt.float32

INV_SCALE = float(np.sqrt(D))  # 8.0, multiplied into q


def _bcast(ap, parts, shape):
    """Broadcast a DRAM AP across `parts` partitions (step-0 partition dim)."""
    return bass.AP(tensor=ap.tensor, offset=ap.offset,
                   ap=[[0, parts]] + list(ap.ap))


def build_program():
    nc = bacc.Bacc("TRN2", target_bir_lowering=False, debug=False,
                   num_devices=NCORES)

    x_in = nc.dram_tensor("x", [R, C], F32, kind="ExternalInput")
    wq_in = nc.dram_tensor("wq", [C, D2], F32, kind="ExternalInput")
    wk_in = nc.dram_tensor("wk", [C, D2], F32, kind="ExternalInput")
    wv_in = nc.dram_tensor("wv", [C, D2], F32, kind="ExternalInput")
    bq_in = nc.dram_tensor("bq", [D2], F32, kind="ExternalInput")
    bk_in = nc.dram_tensor("bk", [D2], F32, kind="ExternalInput")
    bv_in = nc.dram_tensor("bv", [D2], F32, kind="ExternalInput")
    wp_in = nc.dram_tensor("wp", [C, C], F32, kind="ExternalInput")
    bp_in = nc.dram_tensor("bp", [C], F32, kind="ExternalInput")
    out_t = nc.dram_tensor("out", [RS, C], F32, kind="ExternalOutput")

    with tile.TileContext(nc) as tc:
        kernel_body(tc, x_in, wq_in, wk_in, wv_in, bq_in, bk_in, bv_in,
                    wp_in, bp_in, out_t)
    nc.compile()
    return nc


def kernel_body(tc, x_in, wq_in, wk_in, wv_in, bq_in, bk_in, bv_in,
                wp_in, bp_in, out_t):
    nc = tc.nc
    Exp = mybir.ActivationFunctionType.Exp
    Ln = mybir.ActivationFunctionType.Ln
    Ident = mybir.ActivationFunctionType.Identity

    ctx = ExitStack()
    consts = ctx.enter_context(tc.tile_pool(name="consts", bufs=1))
    persist = ctx.enter_context(tc.tile_pool(name="persist", bufs=1))
    dram = ctx.enter_context(tc.tile_pool(name="dram", bufs=1, space="DRAM"))

    ident = consts.tile([128, 128], F32)
    make_identity(nc, ident)

    # --- weights / biases for qkv ---
    wq_sb = consts.tile([128, KS, D2], F32)
    wk_sb = consts.tile([128, KS, D2], F32)
    wv_sb = consts.tile([128, KS, D2], F32)
    nc.sync.dma_start(wq_sb, wq_in.ap().rearrange("(ks p) m -> p ks m", p=128))
    nc.sync.dma_start(wk_sb, wk_in.ap().rearrange("(ks p) m -> p ks m", p=128))
    nc.sync.dma_start(wv_sb, wv_in.ap().rearrange("(ks p) m -> p ks m", p=128))

    bq_sb = consts.tile([128, 1], F32)
    bk_sb = consts.tile([128, 1], F32)
    nc.sync.dma_start(bq_sb, bq_in.ap().rearrange("(p o) -> p o", o=1))
    nc.sync.dma_start(bk_sb, bk_in.ap().rearrange("(p o) -> p o", o=1))
    bq8_sb = consts.tile([128, 1], F32)
    nc.scalar.mul(bq8_sb, bq_sb, INV_SCALE)
    # bv replicated across partitions (DMA may broadcast, DVE may not)
    bv_sb = consts.tile([128, D2], F32)
    nc.sync.dma_start(bv_sb, _bcast(bv_in.ap(), 128, None))

    # --- persistent activations ---
    qT_sb = persist.tile([128, R], F32)       # 8 * q^T  (d2 on partitions)
    kT_sb = persist.tile([128, R], F32)       # k^T
    v_sb = persist.tile([128, R // 128, D2], F32)  # v natural (rows on partitions)
    outT_sb = persist.tile([128, R], F32)     # attention out^T for my heads

    # ---------- Phase 1+2: x^T chunks and QKV projections ----------
    with tc.tile_pool(name="xload", bufs=2) as xload, \
         tc.tile_pool(name="p12", bufs=1, space="PSUM") as p12:
        for ch in range(NCH):
            r0 = ch * CHUNK
            x_nat = xload.tile([128, 4, C], F32, tag="x_nat")
            nc.sync.dma_start(
                x_nat, x_in.ap()[r0:r0 + CHUNK, :].rearrange(
                    "(m p) c -> p m c", p=128))
            xT = xload.tile([128, KS, CHUNK], F32, tag="xT")
            for ks in range(KS):
                for m in range(4):
                    pt = p12.tile([128, 128], F32, tag="pt", bufs=2)
                    nc.tensor.transpose(
                        pt, x_nat[:, m, ks * 128:(ks + 1) * 128], ident)
                    nc.vector.tensor_copy(xT[:, ks, m * 128:(m + 1) * 128], pt)
            # q^T, k^T for this chunk (128 rows = 2 heads * 64)
            pq = p12.tile([128, CHUNK], F32, tag="pq", bufs=2)
            pk = p12.tile([128, CHUNK], F32, tag="pk", bufs=2)
            for ks in range(KS):
                nc.tensor.matmul(pq, wq_sb[:, ks], xT[:, ks],
                                 start=(ks == 0), stop=(ks == KS - 1))
            for ks in range(KS):
                nc.tensor.matmul(pk, wk_sb[:, ks], xT[:, ks],
                                 start=(ks == 0), stop=(ks == KS - 1))
            # qT scaled by sqrt(d) (folds the score scale); biases added
            nc.scalar.activation(qT_sb[:, r0:r0 + CHUNK], pq, Ident,
                                 bias=bq8_sb, scale=INV_SCALE)
            nc.scalar.activation(kT_sb[:, r0:r0 + CHUNK], pk, Ident,
                                 bias=bk_sb, scale=1.0)
            # v natural layout
            for m in range(4):
                pv = p12.tile([128, D2], F32, tag="pv", bufs=2)
                for ks in range(KS):
                    nc.tensor.matmul(pv, xT[:, ks, m * 128:(m + 1) * 128],
                                     wv_sb[:, ks],
                                     start=(ks == 0), stop=(ks == KS - 1))
                nc.vector.tensor_add(v_sb[:, ch * 4 + m, :], pv, bv_sb)

    # ---------- Phase 3: attention per (batch, head) ----------
    with tc.tile_pool(name="att", bufs=1) as att, \
         tc.tile_pool(name="eTp", bufs=2) as eTp, \
         tc.tile_pool(name="p3", bufs=1, space="PSUM") as p3:
        for b in range(B):
            c0 = b * N  # column offset into qT/kT for this batch
            for h in range(HPC):
                hp = h * D  # partition offset of this head
                # --- pass 1 (natural layout): -max and -(max + ln sum) ---
                stats = att.tile([128, 32], F32, tag="stats", bufs=2)
                for mt in range(NQT):
                    q_l = qT_sb[hp:hp + D, c0 + mt * 128: c0 + (mt + 1) * 128]
                    ps = []
                    for j in range(4):
                        p = p3.tile([128, 512], F32, tag="ps", bufs=4)
                        nc.tensor.matmul(
                            p, q_l, kT_sb[hp:hp + D, c0 + j * 512:c0 + (j + 1) * 512],
                            start=True, stop=True)
                        ps.append(p)
                    mx = att.tile([128, 4], F32, tag="mx", bufs=4)
                    for j in range(4):
                        nc.vector.reduce_max(mx[:, j:j + 1], ps[j],
                                             axis=mybir.AxisListType.X)
                    negm = stats[:, mt:mt + 1]
                    nc.vector.reduce_max(negm, mx, axis=mybir.AxisListType.X,
                                         negate=True)
                    esum = att.tile([128, 4], F32, tag="esum", bufs=4)
                    for j in range(4):
                        scr = att.tile([128, 512], F32, tag="scr", bufs=4)
                        nc.scalar.activation(scr, ps[j], Exp, bias=negm,
                                             scale=1.0,
                                             accum_out=esum[:, j:j + 1])
                    ssum = att.tile([128, 1], F32, tag="ssum", bufs=4)
                    nc.vector.reduce_sum(ssum, esum, axis=mybir.AxisListType.X)
                    lns = att.tile([128, 1], F32, tag="lns", bufs=4)
                    nc.scalar.activation(lns, ssum, Ln)
                    # stats[:, 16+mt] = -m - ln(sum)
                    nc.vector.tensor_sub(stats[:, 16 + mt:17 + mt], negm, lns)

                # transpose stats; row of interest: bias = -(m + ln sum) in
                # [1, N] layout (free dim = query index)
                pstat = p3.tile([32, 128], F32, tag="psT", bufs=2)
                nc.tensor.transpose(pstat, stats, ident)
                statsT = att.tile([32, 128], F32, tag="statsT", bufs=2)
                nc.scalar.copy(statsT, pstat)
                biasT = att.tile([1, N], F32, tag="biasT", bufs=2)
                nc.sync.dma_start(
                    biasT.rearrange("s (m q) -> s m q", m=16),
                    statsT[16:32, :])

                # k^T extended with a ones row (bias contraction row)
                kT_ext = att.tile([65, N], F32, tag="kT_ext", bufs=2)
                nc.vector.tensor_copy(kT_ext[0:64, :], kT_sb[hp:hp + D, c0:c0 + N])
                nc.vector.memset(kT_ext[64:65, :], 1.0)

                # --- pass 2 (transposed layout): exp scores, A@V ---
                for j in range(4):
                    q_ext = att.tile([65, 512], F32, tag="q_ext", bufs=2)
                    nc.vector.tensor_copy(
                        q_ext[0:64, :],
                        qT_sb[hp:hp + D, c0 + j * 512:c0 + (j + 1) * 512])
                    nc.vector.tensor_copy(
                        q_ext[64:65, :], biasT[0:1, j * 512:(j + 1) * 512])
                    eT = eTp.tile([128, NKT, 512], F32, tag="eT")
                    for kt in range(NKT):
                        psT = p3.tile([128, 512], F32, tag="psT", bufs=2)
                        nc.tensor.matmul(
                            psT, kT_ext[:, kt * 128:(kt + 1) * 128],
                            q_ext, start=True, stop=True)
                        nc.scalar.activation(eT[:, kt], psT, Exp)
                    pav = p3.tile([64, 512], F32, tag="pav", bufs=2)
                    for kt in range(NKT):
                        nc.tensor.matmul(pav,
                                         v_sb[:, b * 16 + kt, hp:hp + D],
                                         eT[:, kt],
                                         start=(kt == 0), stop=(kt == NKT - 1))
                    nc.vector.tensor_copy(
                        outT_sb[hp:hp + D, c0 + j * 512:c0 + (j + 1) * 512],
                        pav)

    # ---------- Phase 4: AllToAll + output projection ----------
    a2a_in = dram.tile([NCORES * 128, RS], F32)
    a2a_out = dram.tile([NCORES * 128, RS], F32)
    nc.sync.dma_start(
        a2a_in.rearrange("(j p) r -> p j r", j=NCORES),
        outT_sb.rearrange("p (j r) -> p j r", j=NCORES))
    nc.gpsimd.collective_compute(
        "AllToAll", mybir.AluOpType.bypass,
        replica_groups=[list(range(NCORES))],
        ins=[a2a_in[:]], outs=[a2a_out[:]])

    with tc.tile_pool(name="proj", bufs=1) as proj, \
         tc.tile_pool(name="p4", bufs=1, space="PSUM") as p4:
        lhsT_proj = proj.tile([128, KS, RS], F32)
        nc.sync.dma_start(lhsT_proj,
                          a2a_out.rearrange("(j p) r -> p j r", j=NCORES))
        wp_sb = proj.tile([128, KS, C], F32)
        nc.sync.dma_start(wp_sb, wp_in.ap().rearrange("(ks p) n -> p ks n", p=128))
        bp_sb = proj.tile([128, C], F32)
        nc.sync.dma_start(bp_sb, _bcast(bp_in.ap(), 128, None))
        for m in range(RS // 128):
            for nt in range(C // 512):
                pp = p4.tile([128, 512], F32, tag="pp", bufs=4)
                for ks in range(KS):
                    nc.tensor.matmul(pp, lhsT_proj[:, ks, m * 128:(m + 1) * 128],
                                     wp_sb[:, ks, nt * 512:(nt + 1) * 512],
                                     start=(ks == 0), stop=(ks == KS - 1))
                o_sb = proj.tile([128, 512], F32, tag="o_sb", bufs=4)
                nc.vector.tensor_add(o_sb, pp,
                                     bp_sb[:, nt * 512:(nt + 1) * 512])
                nc.sync.dma_start(
                    out_t.ap()[m * 128:(m + 1) * 128, nt * 512:(nt + 1) * 512],
                    o_sb)
    ctx.close()


_PROGRAM = None


def _get_program():
    global _PROGRAM
    if _PROGRAM is None:
        _PROGRAM = build_program()
    return _PROGRAM


def kernel(x, W_qkv, b_qkv, W_proj, b_proj, _trace=False):
    x = np.ascontiguousarray(np.asarray(x, dtype=np.float32).reshape(R, C))
    W_qkv = np.asarray(W_qkv, dtype=np.float32)
    b_qkv = np.asarray(b_qkv, dtype=np.float32)
    W_proj = np.ascontiguousarray(np.asarray(W_proj, dtype=np.float32))
    b_proj = np.ascontiguousarray(np.asarray(b_proj, dtype=np.float32))

    in_maps = []
    for i in range(NCORES):
        lo = i * D2            # first column of my heads within a qkv block
        hi = lo + D2
        in_maps.append({
            "x": x,
            "wq": np.ascontiguousarray(W_qkv[:, 0 * C + lo:0 * C + hi]),
            "wk": np.ascontiguousarray(W_qkv[:, 1 * C + lo:1 * C + hi]),
            "wv": np.ascontiguousarray(W_qkv[:, 2 * C + lo:2 * C + hi]),
            "bq": np.ascontiguousarray(b_qkv[0 * C + lo:0 * C + hi]),
            "bk": np.ascontiguousarray(b_qkv[1 * C + lo:1 * C + hi]),
            "bv": np.ascontiguousarray(b_qkv[2 * C + lo:2 * C + hi]),
            "wp": W_proj,
            "bp": b_proj,
        })

    nc = _get_program()
    res = bass_utils.run_bass_kernel_spmd(
        nc, in_maps, core_ids=list(range(NCORES)), trace=_trace)
    out = np.concatenate([res.results[i]["out"] for i in range(NCORES)], axis=0)
    if _trace:
        kernel.last_results = res
    return out.reshape(B, N, C)


# revision 6
# speedup vs baseline: 2.2106x; 2.2106x over previous
"""Trainium2 Bass kernel for 16-head MHA (b=2, n=2048, c=1024, d=64).

Reference semantics (note the inverted scale "bug" reproduced faithfully):
    qkv = x @ W_qkv + b_qkv
    scores = (q @ k^T) * sqrt(d)          # multiplied, not divided
    out = softmax(scores) @ v
    y = concat_heads(out) @ W_proj + b_proj

Sharding: tensor-parallel over heads. Each of the 8 cores computes QKV +
attention for its 2 heads (head-dim-transposed layouts so no activation
transposes are needed beyond one x^T pass), then an AllToAll moves the
per-head attention outputs into a row-sharded layout and each core computes
the final projection for its 512-row output shard. Host concatenates shards.

Precision strategy: exactness is needed only upstream of exp (the *sqrt(d)
score scale amplifies absolute errors into exp-space relative errors).
Those matmuls use a bf16 hi/lo split, 3 accumulating passes
(hi*hi + hi*lo + lo*hi; the dropped lo*lo term is ~2^-18 relative) —
products are exact in the PE and accumulate in fp32, giving ~1e-3 absolute
score accuracy at 3x the bf16 rate instead of fp32's 4x penalty.
The row-max pass needs no precision at all (softmax subtracts and the
explicit renormalization cancels any max offset exactly), so it runs
single-pass bf16. exp scores, A@V, and the projection are plain bf16.

Softmax plumbing: the transposed-score matmul gets an extra contraction
row (ones in k^T_ext, -rowmax in q_ext) so exp(psum) needs no separate
bias op; V gets an extra ones *column* so the A@V matmul also yields the
softmax denominators, and one reciprocal + gpsimd partition-broadcast +
multiply normalizes the head output.
"""

import sys
from contextlib import ExitStack

sys.path.insert(0, "/opt/trn_rl_repo")

import numpy as np

import concourse.bass as bass
import concourse.tile as tile
from concourse import bacc, mybir
from concourse import bass_utils
from concourse.masks import make_identity

# Problem shape (hardcoded per contract)
B, N, C = 2, 2048, 1024
H, D = 16, 64
NCORES = 8
HPC = H // NCORES          # heads per core = 2
D2 = HPC * D               # 128 = per-core slice of the concat dim
R = B * N                  # 4096 flattened rows
RS = R // NCORES           # 512 output rows per core
KS = C // 128              # 8 contraction blocks of 128
CHUNK = 512                # rows per x^T/qkv chunk
NCH = R // CHUNK           # 8 chunks
NQT = N // 128             # 16 query tiles per batch
NKT = N // 128             # 16 key tiles per batch
F32 = mybir.dt.float32
BF16 = mybir.dt.float16  # fp16: same PE rate as bf16, 8x the mantissa

INV_SCALE = float(np.sqrt(D))  # 8.0, multiplied into q


def _bcast(ap, parts):
    """Broadcast a DRAM AP across `parts` partitions (step-0 partition dim)."""
    return bass.AP(tensor=ap.tensor, offset=ap.offset,
                   ap=[[0, parts]] + list(ap.ap))


def build_program():
    nc = bacc.Bacc("TRN2", target_bir_lowering=False, debug=False,
                   num_devices=NCORES)

    x_in = nc.dram_tensor("x", [R, C], F32, kind="ExternalInput")
    wq_in = nc.dram_tensor("wq", [C, D2], F32, kind="ExternalInput")
    wk_in = nc.dram_tensor("wk", [C, D2], F32, kind="ExternalInput")
    wv_in = nc.dram_tensor("wv", [C, D2], F32, kind="ExternalInput")
    bq_in = nc.dram_tensor("bq", [D2], F32, kind="ExternalInput")
    bk_in = nc.dram_tensor("bk", [D2], F32, kind="ExternalInput")
    bv_in = nc.dram_tensor("bv", [D2], F32, kind="ExternalInput")
    wp_in = nc.dram_tensor("wp", [C, C], F32, kind="ExternalInput")
    bp_in = nc.dram_tensor("bp", [C], F32, kind="ExternalInput")
    out_t = nc.dram_tensor("out", [RS, C], F32, kind="ExternalOutput")

    with tile.TileContext(nc) as tc:
        kernel_body(tc, x_in, wq_in, wk_in, wv_in, bq_in, bk_in, bv_in,
                    wp_in, bp_in, out_t)
    nc.compile()
    return nc


def _split_hi_lo(nc, pool, src_ap, shape, tag):
    """bf16 hi/lo split of an fp32 SBUF AP: hi = bf16(x), lo = bf16(x - hi)."""
    hi = pool.tile(shape, BF16, tag=tag + "_hi", name=tag + "_hi")
    lo = pool.tile(shape, BF16, tag=tag + "_lo", name=tag + "_lo")
    nc.vector.tensor_copy(hi, src_ap)
    nc.vector.tensor_sub(lo, src_ap, hi)
    return hi, lo


def kernel_body(tc, x_in, wq_in, wk_in, wv_in, bq_in, bk_in, bv_in,
                wp_in, bp_in, out_t):
    nc = tc.nc
    Exp = mybir.ActivationFunctionType.Exp
    Ident = mybir.ActivationFunctionType.Identity

    ctx = ExitStack()
    consts = ctx.enter_context(tc.tile_pool(name="consts", bufs=1))
    persist = ctx.enter_context(tc.tile_pool(name="persist", bufs=1))
    dram = ctx.enter_context(tc.tile_pool(name="dram", bufs=1, space="DRAM"))

    ident = consts.tile([128, 128], F32)
    make_identity(nc, ident)

    # --- weights / biases for qkv (hi/lo split in bf16) ---
    wsplit = {}
    for name, t_in in (("wq", wq_in), ("wk", wk_in), ("wv", wv_in)):
        w_f32 = consts.tile([128, KS, D2], F32, name=name + "_f32")
        nc.sync.dma_start(w_f32, t_in.ap().rearrange("(ks p) m -> p ks m", p=128))
        if name == "wv":
            wv_bf = consts.tile([128, KS, D2], BF16, name="wv_bf")
            nc.vector.tensor_copy(wv_bf, w_f32)
            wsplit[name] = (wv_bf, None)
        else:
            wsplit[name] = _split_hi_lo(nc, consts, w_f32, [128, KS, D2], name)
    wq_hi, wq_lo = wsplit["wq"]
    wk_hi, wk_lo = wsplit["wk"]
    wv_bf = wsplit["wv"][0]

    bq_sb = consts.tile([128, 1], F32)
    bk_sb = consts.tile([128, 1], F32)
    nc.sync.dma_start(bq_sb, bq_in.ap().rearrange("(p o) -> p o", o=1))
    nc.sync.dma_start(bk_sb, bk_in.ap().rearrange("(p o) -> p o", o=1))
    bq8_sb = consts.tile([128, 1], F32)
    nc.scalar.mul(bq8_sb, bq_sb, INV_SCALE)
    bv_sb = consts.tile([128, D2], F32)
    nc.sync.dma_start(bv_sb, _bcast(bv_in.ap(), 128))

    # --- persistent activations (all bf16) ---
    qT_hi = persist.tile([128, R], BF16)   # sqrt(d) * q^T, hi part
    qT_lo = persist.tile([128, R], BF16)
    kT_hi = persist.tile([128, R], BF16)
    kT_lo = persist.tile([128, R], BF16)
    # v with a ones column per head: [p, row_tile, head, 65]
    v_sb = persist.tile([128, R // 128, HPC, D + 1], BF16)
    nc.vector.memset(v_sb[:, :, :, D:D + 1], 1.0)
    outT_sb = persist.tile([128, R], BF16)

    # ---------- Phase 1+2: x^T chunks and QKV projections ----------
    with tc.tile_pool(name="xload", bufs=2) as xload, \
         tc.tile_pool(name="p12", bufs=1, space="PSUM") as p12:
        for ch in range(NCH):
            r0 = ch * CHUNK
            x_nat = xload.tile([128, 4, C], F32, tag="x_nat")
            nc.sync.dma_start(
                x_nat, x_in.ap()[r0:r0 + CHUNK, :].rearrange(
                    "(m p) c -> p m c", p=128))
            xT = xload.tile([128, KS, CHUNK], F32, tag="xT")
            for ks in range(KS):
                for m in range(4):
                    pt = p12.tile([128, 128], F32, tag="pt", bufs=2)
                    nc.tensor.transpose(
                        pt, x_nat[:, m, ks * 128:(ks + 1) * 128], ident)
                    nc.vector.tensor_copy(xT[:, ks, m * 128:(m + 1) * 128], pt)
            xT_hi, xT_lo = _split_hi_lo(nc, xload, xT, [128, KS, CHUNK], "xTs")

            # q^T, k^T for this chunk (128 rows = 2 heads * 64), 3-pass split
            for (whi, wlo, dst_hi, dst_lo, bias, scale) in (
                    (wq_hi, wq_lo, qT_hi, qT_lo, bq8_sb, INV_SCALE),
                    (wk_hi, wk_lo, kT_hi, kT_lo, bk_sb, 1.0)):
                pqk = p12.tile([128, CHUNK], F32, tag="pqk", bufs=2)
                passes = [(whi, xT_hi), (whi, xT_lo), (wlo, xT_hi)]
                for pi, (w_p, x_p) in enumerate(passes):
                    for ks in range(KS):
                        nc.tensor.matmul(pqk, w_p[:, ks], x_p[:, ks],
                                         start=(pi == 0 and ks == 0),
                                         stop=(pi == 2 and ks == KS - 1))
                # hi = bf16(scale*psum + bias); lo = bf16((scale*psum+bias) - hi)
                nc.scalar.activation(dst_hi[:, r0:r0 + CHUNK], pqk, Ident,
                                     bias=bias, scale=scale)
                tmp = xload.tile([128, CHUNK], F32, tag="qk_tmp", bufs=2)
                nc.vector.tensor_scalar(tmp, pqk, scalar1=scale, scalar2=bias,
                                        op0=mybir.AluOpType.mult,
                                        op1=mybir.AluOpType.add)
                nc.vector.tensor_sub(dst_lo[:, r0:r0 + CHUNK], tmp,
                                     dst_hi[:, r0:r0 + CHUNK])

            # v natural layout (single-pass bf16)
            for m in range(4):
                pv = p12.tile([128, D2], F32, tag="pv", bufs=2)
                for ks in range(KS):
                    nc.tensor.matmul(pv, xT_hi[:, ks, m * 128:(m + 1) * 128],
                                     wv_bf[:, ks],
                                     start=(ks == 0), stop=(ks == KS - 1))
                for h in range(HPC):
                    nc.vector.tensor_add(
                        v_sb[:, ch * 4 + m, h, 0:D],
                        pv[:, h * D:(h + 1) * D],
                        bv_sb[:, h * D:(h + 1) * D])

    # ---------- Phase 3: attention per (batch, head) ----------
    with tc.tile_pool(name="att", bufs=1) as att, \
         tc.tile_pool(name="eTp", bufs=2) as eTp, \
         tc.tile_pool(name="p3", bufs=1, space="PSUM") as p3:
        for b in range(B):
            c0 = b * N  # column offset into qT/kT for this batch
            for h in range(HPC):
                hp = h * D  # partition offset of this head
                # --- pass 1 (natural layout, single bf16): -rowmax ---
                stats = att.tile([128, 16], F32, tag="stats", bufs=2)
                for mt in range(NQT):
                    q_l = qT_hi[hp:hp + D, c0 + mt * 128: c0 + (mt + 1) * 128]
                    ps = []
                    for j in range(4):
                        p = p3.tile([128, 512], F32, tag="ps", bufs=4)
                        nc.tensor.matmul(
                            p, q_l,
                            kT_hi[hp:hp + D, c0 + j * 512:c0 + (j + 1) * 512],
                            start=True, stop=True)
                        ps.append(p)
                    mx = att.tile([128, 4], F32, tag="mx", bufs=4)
                    for j in range(4):
                        nc.vector.reduce_max(mx[:, j:j + 1], ps[j],
                                             axis=mybir.AxisListType.X)
                    nc.vector.reduce_max(stats[:, mt:mt + 1], mx,
                                         axis=mybir.AxisListType.X, negate=True)

                # transpose -rowmax into [1, N] (free = query index)
                pstat = p3.tile([16, 128], F32, tag="psT", bufs=2)
                nc.tensor.transpose(pstat, stats, ident)
                statsT = att.tile([16, 128], F32, tag="statsT", bufs=2)
                nc.vector.tensor_copy(statsT, pstat)
                biasT = att.tile([1, N], F32, tag="biasT", bufs=2)
                nc.sync.dma_start(
                    biasT.rearrange("s (m q) -> s m q", m=16), statsT)

                # k^T extended with bias contraction row (ones / zeros)
                kThx = att.tile([65, N], BF16, tag="kThx", bufs=2)
                kTlx = att.tile([65, N], BF16, tag="kTlx", bufs=2)
                nc.vector.tensor_copy(kThx[0:64, :], kT_hi[hp:hp + D, c0:c0 + N])
                nc.vector.memset(kThx[64:65, :], 1.0)
                nc.vector.tensor_copy(kTlx[0:64, :], kT_lo[hp:hp + D, c0:c0 + N])
                nc.vector.memset(kTlx[64:65, :], 0.0)

                # --- pass 2 (transposed): exp scores then A@V ---
                for j in range(4):
                    qs = slice(c0 + j * 512, c0 + (j + 1) * 512)
                    qhx = att.tile([65, 512], BF16, tag="qhx", bufs=2)
                    qlx = att.tile([65, 512], BF16, tag="qlx", bufs=2)
                    nc.vector.tensor_copy(qhx[0:64, :], qT_hi[hp:hp + D, qs])
                    nc.vector.tensor_copy(
                        qhx[64:65, :], biasT[0:1, j * 512:(j + 1) * 512])
                    nc.vector.tensor_copy(qlx[0:64, :], qT_lo[hp:hp + D, qs])
                    nc.vector.memset(qlx[64:65, :], 0.0)
                    eT = eTp.tile([128, NKT, 512], BF16, tag="eT")
                    for kt in range(NKT):
                        psT = p3.tile([128, 512], F32, tag="psT", bufs=2)
                        kslc = slice(kt * 128, (kt + 1) * 128)
                        nc.tensor.matmul(psT, kThx[:, kslc], qhx,
                                         start=True, stop=False)
                        nc.tensor.matmul(psT, kThx[:, kslc], qlx,
                                         start=False, stop=False)
                        nc.tensor.matmul(psT, kTlx[:, kslc], qhx,
                                         start=False, stop=True)
                        nc.scalar.activation(eT[:, kt], psT, Exp)
                    pav = p3.tile([65, 512], F32, tag="pav", bufs=2)
                    for kt in range(NKT):
                        nc.tensor.matmul(pav, v_sb[:, b * 16 + kt, h, :],
                                         eT[:, kt],
                                         start=(kt == 0), stop=(kt == NKT - 1))
                    # normalize: row 64 of pav holds the softmax denominators
                    rj = att.tile([1, 512], F32, tag="rj", bufs=2)
                    nc.vector.reciprocal(rj, pav[64:65, :])
                    rrep = att.tile([64, 512], F32, tag="rrep", bufs=2)
                    nc.gpsimd.partition_broadcast(rrep, rj)
                    nc.vector.tensor_mul(outT_sb[hp:hp + D, qs],
                                         pav[0:64, :], rrep)

    # ---------- Phase 4: AllToAll + output projection ----------
    a2a_in = dram.tile([NCORES * 128, RS], BF16)
    a2a_out = dram.tile([NCORES * 128, RS], BF16)
    nc.sync.dma_start(
        a2a_in.rearrange("(j p) r -> p j r", j=NCORES),
        outT_sb.rearrange("p (j r) -> p j r", j=NCORES))
    nc.gpsimd.collective_compute(
        "AllToAll", mybir.AluOpType.bypass,
        replica_groups=[list(range(NCORES))],
        ins=[a2a_in[:]], outs=[a2a_out[:]])

    with tc.tile_pool(name="proj", bufs=1) as proj, \
         tc.tile_pool(name="p4", bufs=1, space="PSUM") as p4:
        lhsT_proj = proj.tile([128, KS, RS], BF16)
        nc.sync.dma_start(lhsT_proj,
                          a2a_out.rearrange("(j p) r -> p j r", j=NCORES))
        wp_f32 = proj.tile([128, KS, C], F32)
        nc.sync.dma_start(wp_f32, wp_in.ap().rearrange("(ks p) n -> p ks n", p=128))
        wp_bf = proj.tile([128, KS, C], BF16)
        nc.vector.tensor_copy(wp_bf, wp_f32)
        bp_sb = proj.tile([128, C], F32)
        nc.sync.dma_start(bp_sb, _bcast(bp_in.ap(), 128))
        for m in range(RS // 128):
            for nt in range(C // 512):
                pp = p4.tile([128, 512], F32, tag="pp", bufs=4)
                for ks in range(KS):
                    nc.tensor.matmul(pp, lhsT_proj[:, ks, m * 128:(m + 1) * 128],
                                     wp_bf[:, ks, nt * 512:(nt + 1) * 512],
                                     start=(ks == 0), stop=(ks == KS - 1))
                o_sb = proj.tile([128, 512], F32, tag="o_sb", bufs=4)
                nc.vector.tensor_add(o_sb, pp,
                                     bp_sb[:, nt * 512:(nt + 1) * 512])
                nc.sync.dma_start(
                    out_t.ap()[m * 128:(m + 1) * 128, nt * 512:(nt + 1) * 512],
                    o_sb)
    ctx.close()


_PROGRAM = None


def _get_program():
    global _PROGRAM
    if _PROGRAM is None:
        _PROGRAM = build_program()
    return _PROGRAM


def kernel(x, W_qkv, b_qkv, W_proj, b_proj, _trace=False):
    x = np.ascontiguousarray(np.asarray(x, dtype=np.float32).reshape(R, C))
    W_qkv = np.asarray(W_qkv, dtype=np.float32)
    b_qkv = np.asarray(b_qkv, dtype=np.float32)
    W_proj = np.ascontiguousarray(np.asarray(W_proj, dtype=np.float32))
    b_proj = np.ascontiguousarray(np.asarray(b_proj, dtype=np.float32))

    in_maps = []
    for i in range(NCORES):
        lo = i * D2            # first column of my heads within a qkv block
        hi = lo + D2
        in_maps.append({
            "x": x,
            "wq": np.ascontiguousarray(W_qkv[:, 0 * C + lo:0 * C + hi]),
            "wk": np.ascontiguousarray(W_qkv[:, 1 * C + lo:1 * C + hi]),
            "wv": np.ascontiguousarray(W_qkv[:, 2 * C + lo:2 * C + hi]),
            "bq": np.ascontiguousarray(b_qkv[0 * C + lo:0 * C + hi]),
            "bk": np.ascontiguousarray(b_qkv[1 * C + lo:1 * C + hi]),
            "bv": np.ascontiguousarray(b_qkv[2 * C + lo:2 * C + hi]),
            "wp": W_proj,
            "bp": b_proj,
        })

    nc = _get_program()
    res = bass_utils.run_bass_kernel_spmd(
        nc, in_maps, core_ids=list(range(NCORES)), trace=_trace)
    out = np.concatenate([res.results[i]["out"] for i in range(NCORES)], axis=0)
    if _trace:
        kernel.last_results = res
    return out.reshape(B, N, C)
